# revision 6
# baseline (speedup 1.0000x reference)
"""Trainium2 Bass kernel for nn_LossSoftDice (soft-dice loss over 32 samples
of 1x512x512 probability/target maps).

Strategy: pure data parallel over the batch. Each of the 8 NeuronCores gets 4
samples (each sample = 262144 f32 elements, viewed as a [128, 2048] tile).
The device computes only per-partition statistics (everything else is
O(128) work done on host during the gather/unshard step).

Per-sample tile md = [m2 | m1] ([128, 4096], m2 = targets in the low half so
one ACT pass can span both halves). Engine balance (measured rates: DVE
1.12 ns/col, ACT 0.92 ns/col + 278 ns/accum, PE fp32 ~2.7 ns/col):
  DVE : maxp[p]  = max_f m2[p,f]            (tensor_reduce)
        inter[p] = sum_f m1[p,f]*m2[p,f]    (scalar_tensor_tensor accum)
        + final [4,512] PSUM->stats reduce of the PE partial sums
  ACT : sgn[p]   = sum_f sign(m1[p,f]-0.5)  (Sign w/ accum; nsr=(N+sgn)/2,
                                             0.5-ties fixed up on host)
        denA[p]  = sum md[p, 768:3328]      (Copy w/ accum, middle band)
  PE  : denP[s]  = sum md[p, outer bands]   (ones-column stationary matmuls
                                             accumulated into PSUM [4,512])
  DMA : m2 on the sync HWDGE queue, m1 on the gpsimd SWDGE queue (sample
        granularity, 8 KB lines); the queues round-robin at packet
        granularity for ~435 GB/s combined.

Host combine (exact, matches the reference's acc branch):
  den = denA + denP;  score = 2*(inter+1)/(den+1)
  corr_b = N - nSR - K + 2A with K (#elements == global max) and A (#those
  with m1 > 0.5) recovered by scanning only the partitions that attain the
  global max (O(2048) per sample, exact); score = 1 where corr == 1;
  loss = mean(1 - score)
"""

import os
import sys
import types

import numpy as np


def _ensure_concourse():
    try:
        import concourse.bass  # noqa: F401
    except ImportError:
        for p in ("/opt/trn_rl_repo", "/root/.axon_site/_ro/trn_rl_repo"):
            if os.path.isdir(p) and p not in sys.path:
                sys.path.insert(0, p)
        import concourse.bass  # noqa: F401


_ensure_concourse()

import concourse.bass as bass  # noqa: E402
import concourse.bacc as bacc  # noqa: E402
import concourse.tile as tile  # noqa: E402
from concourse import mybir  # noqa: E402
from concourse.bass_utils import run_bass_kernel_spmd  # noqa: E402
from concourse.vector_clock import ScopedClock  # noqa: E402

N_CORES = 8
B = 32                      # total batch
BPC = B // N_CORES          # samples per core
P = 128                     # partitions
F = 2048                    # free dim per partition (P*F = 512*512)

# den column split of the [m2|m1] combined tile (4096 cols):
# PE takes the outer bands [0, PE_LO) and [4096-PE_HI, 4096); ACT takes the
# contiguous middle band. PE_LO/PE_HI multiples of 512 (one matmul each).
PE_LO = 1024
PE_HI = 512
ACT_BAND = (PE_LO, 2 * F - PE_HI)

# sample chunking: samples 0-2 as full [128,2048] transfers, sample 3 split
# in two 1024-col chunks so the post-stream compute tail is shorter
SLOTS = [(0, 0, F), (1, 0, F), (2, 0, F), (3, 0, 1024), (3, 1024, 1024)]
NSL = len(SLOTS)


def _slim_drain_and_barrier(self, tick_clock, wait_clock):
    # Same as TileContext._drain_and_barrier but without the second
    # all-engine barrier: NRT itself waits for every engine to halt before
    # the NEFF can be re-executed, so the sem clear does not need another
    # intra-NEFF barrier after it. (Bacc.compile legalizes multi-waits.)
    nc = self.nc
    drain_inst = nc.sync.drain()
    wait_clock.add_sem_waits(
        drain_inst.ins, ScopedClock({None: tick_clock.global_clock})
    )
    nc.all_engine_barrier()
    assert self.sems is not None
    popped = nc._tile_sem_poison_stack.pop()
    assert popped is self._sem_poison
    nc.clear_and_free_semaphores(list(self.sems.allocated().values()))


tile.TileContext._drain_and_barrier = _slim_drain_and_barrier


def _install_ntff_hook_module():
    """bass_utils imports antenv.axon_hooks when trace=True under axon; this
    container's antenv lacks that module. Recreate it from the boot helper."""
    if "antenv.axon_hooks" in sys.modules:
        return
    try:
        import trn_agent_boot.trn_boot as tb

        hook = tb._ntff_profile_via_ctypes("/opt/axon/libaxon_pjrt.so")
    except Exception:
        hook = None
    m = types.ModuleType("antenv.axon_hooks")
    m.get_axon_ntff_profile_hook = lambda: hook
    m.set_axon_ntff_profile_hook = lambda h: None
    sys.modules["antenv.axon_hooks"] = m


# stats tile layout (f32 columns)
COL_MAXP = 0
COL_INTER = NSL
COL_SGN = 2 * NSL
COL_DENA = 3 * NSL
COL_DENP = 4 * NSL          # rows 0..3 hold the PE den partial per sample
N_COLS = 4 * NSL + 1


def _build_nc():
    nc = bacc.Bacc("TRN2", debug=False)
    f32 = mybir.dt.float32
    probs = nc.dram_tensor("probs", [BPC, P, F], f32, kind="ExternalInput").ap()
    targets = nc.dram_tensor("targets", [BPC, P, F], f32, kind="ExternalInput").ap()
    stats_out = nc.dram_tensor("stats", [P, N_COLS], f32, kind="ExternalOutput").ap()

    A = mybir.AluOpType
    AF = mybir.ActivationFunctionType
    with tile.TileContext(nc) as tc:
        with (
            tc.tile_pool(name="inp", bufs=1) as inp_pool,
            tc.tile_pool(name="scr", bufs=1) as scr_pool,
            tc.tile_pool(name="stats", bufs=1) as stats_pool,
            tc.psum_pool(name="psum", bufs=1) as psum_pool,
        ):
            mds = [
                inp_pool.tile([P, 2 * F], f32, tag=f"md{s}", name=f"md{s}")
                for s in range(BPC)
            ]

            ones = scr_pool.tile([P, 4 * BPC], f32, tag="ones")
            neg_half = scr_pool.tile([P, 1], f32, tag="neg_half")
            dve_scr = scr_pool.tile([P, F], f32, tag="dve_scr")
            sgn_scr = scr_pool.tile([P, F], f32, tag="sgn_scr")
            cp_scr = scr_pool.tile([P, 2 * F - PE_LO - PE_HI], f32, tag="cp_scr")
            st = stats_pool.tile([P, N_COLS], f32, tag="st", name="st_all")
            psum_t = psum_pool.tile([BPC, 512], f32, tag="acc")

            # input DMAs first so the gpsimd SWDGE queue starts streaming
            # before the (gpsimd) constant memsets: m2 -> sync HWDGE (low
            # half), m1 -> gpsimd SWDGE (high half); 8KB per-partition lines
            for s, c0, w in SLOTS:
                nc.sync.dma_start(
                    mds[s][:, c0 : c0 + w], targets[s][:, c0 : c0 + w]
                )
                nc.gpsimd.dma_start(
                    mds[s][:, F + c0 : F + c0 + w], probs[s][:, c0 : c0 + w]
                )

            # stationary weights: cols 4s+s hold 1.0 so lhsT slice
            # [:, 4s:4s+4] routes sample s's column sums to PSUM row s
            nc.gpsimd.memset(ones[:], 0.0)
            for s in range(BPC):
                nc.gpsimd.memset(ones[:, 4 * s + s : 4 * s + s + 1], 1.0)
            nc.gpsimd.memset(neg_half[:], -0.5)

            n_mm = 0
            total_mm = BPC * (PE_LO + PE_HI) // 512
            for i, (s, c0, w) in enumerate(SLOTS):
                md = mds[s]
                m2 = md[:, c0 : c0 + w]
                m1 = md[:, F + c0 : F + c0 + w]
                # DVE: per-partition max of targets
                nc.vector.tensor_reduce(
                    st[:, COL_MAXP + i : COL_MAXP + i + 1],
                    m2,
                    mybir.AxisListType.X,
                    A.max,
                )
                # ACT: count(m1 > 0.5) via sign(m1 - 0.5) accumulation
                nc.scalar.activation(
                    sgn_scr[:, 0:w], m1, AF.Sign, bias=neg_half[:],
                    accum_out=st[:, COL_SGN + i : COL_SGN + i + 1],
                )
                # DVE: intersection accumulate
                nc.vector.scalar_tensor_tensor(
                    out=dve_scr[:, 0:w],
                    in0=m1,
                    scalar=1.0,
                    in1=m2,
                    op0=A.mult,
                    op1=A.mult,
                    accum_out=st[:, COL_INTER + i : COL_INTER + i + 1],
                )
                if c0 == 0:
                    # once per sample: ACT middle-band den + PE outer bands
                    nc.scalar.activation(
                        cp_scr[:],
                        md[:, ACT_BAND[0] : ACT_BAND[1]],
                        AF.Copy,
                        accum_out=st[:, COL_DENA + i : COL_DENA + i + 1],
                    )
                    for q0 in list(range(0, PE_LO, 512)) + list(
                        range(2 * F - PE_HI, 2 * F, 512)
                    ):
                        nc.tensor.matmul(
                            psum_t[:, :],
                            ones[:, 4 * s : 4 * s + 4],
                            md[:, q0 : q0 + 512],
                            start=(n_mm == 0),
                            stop=(n_mm == total_mm - 1),
                            skip_group_check=True,
                        )
                        n_mm += 1

            # DVE: fold PE partial sums -> per-sample den part in rows 0..3
            nc.vector.tensor_reduce(
                st[0:BPC, COL_DENP : COL_DENP + 1],
                psum_t[:, :],
                mybir.AxisListType.X,
                A.add,
            )

            nc.sync.dma_start(stats_out, st[:])

    nc.compile()
    return nc


def _shard_inputs(probs, targets):
    probs = np.ascontiguousarray(np.asarray(probs, dtype=np.float32)).reshape(B, P, F)
    targets = np.ascontiguousarray(np.asarray(targets, dtype=np.float32)).reshape(
        B, P, F
    )
    in_maps = []
    for i in range(N_CORES):
        sl = slice(i * BPC, (i + 1) * BPC)
        in_maps.append(
            {
                "probs": np.ascontiguousarray(probs[sl]),
                "targets": np.ascontiguousarray(targets[sl]),
            }
        )
    return in_maps


def _combine(results, probs, targets):
    """Exact host-side combine of per-partition stats -> scalar loss."""
    inter = np.empty(B)
    den = np.empty(B)
    corr = np.empty(B)
    N = float(P * F)
    for i in range(N_CORES):
        r = results[i]["stats"].astype(np.float64)
        for s in range(BPC):
            b = i * BPC + s
            idx = [j for j, (cs, _, _) in enumerate(SLOTS) if cs == s]
            inter[b] = r[:, [COL_INTER + j for j in idx]].sum()
            # den: ACT middle band (one slot per sample, at the c0==0 slot)
            # plus the PE outer-band partial in row s
            j0 = idx[0]
            den[b] = r[:, COL_DENA + j0].sum() + r[s, COL_DENP]
            # nsr from sign sums; exact-0.5 ties make (w + S) odd -> rescan
            nsr = 0.0
            for j in idx:
                cs, c0, w = SLOTS[j]
                tot = w + r[:, COL_SGN + j]
                odd = np.nonzero(np.round(tot).astype(np.int64) & 1)[0]
                for p in odd:
                    tot[p] = 2 * np.count_nonzero(probs[b, p, c0 : c0 + w] > 0.5)
                nsr += tot.sum() / 2.0
            maxp = r[:, [COL_MAXP + j for j in idx]].max(axis=1)
            gmax = maxp.max()
            K = Acnt = 0
            for p in np.nonzero(maxp == gmax)[0]:
                hit = targets[b, p, :] == np.float32(gmax)
                K += int(hit.sum())
                Acnt += int((hit & (probs[b, p, :] > 0.5)).sum())
            corr[b] = N - nsr - K + 2 * Acnt
    score = 2.0 * (inter + 1.0) / (den + 1.0)
    score = np.where(corr == 1.0, 1.0, score)
    return np.array(np.mean(1.0 - score), dtype=np.float32)


def _run(probs, targets, trace=False, tmpdir=None):
    _install_ntff_hook_module()
    nc = _build_nc()
    in_maps = _shard_inputs(probs, targets)
    res = run_bass_kernel_spmd(
        nc, in_maps, list(range(N_CORES)), trace=trace, tmpdir=tmpdir
    )
    pr = np.asarray(probs, dtype=np.float32).reshape(B, P, F)
    tg = np.asarray(targets, dtype=np.float32).reshape(B, P, F)
    out = _combine(res.results, pr, tg)
    return out, res


def kernel(probs, targets):
    out, _ = _run(probs, targets)
    return out


# revision 11
# speedup vs baseline: 1.2059x; 1.2059x over previous
"""Trainium2 Bass kernel for nn_LossSoftDice (soft-dice loss over 32 samples
of 1x512x512 probability/target maps).

Strategy: pure data parallel over the batch. Each of the 8 NeuronCores gets 4
samples (each sample = 262144 f32 elements, viewed as a [128, 2048] tile).
The device computes only per-partition statistics (everything else is
O(128) work done on host during the gather/unshard step).

Per-sample tile md = [m2 | m1] ([128, 4096], m2 = targets in the low half so
one ACT pass can span both halves). Engine balance (measured rates: DVE
1.12 ns/col, ACT 0.92 ns/col + 278 ns/accum, PE fp32 ~2.7 ns/col):
  DVE : maxp[p]  = max_f m2[p,f]            (tensor_reduce)
        inter[p] = sum_f m1[p,f]*m2[p,f]    (scalar_tensor_tensor accum)
        + final [4,512] PSUM->stats reduce of the PE partial sums
  ACT : sgn[p]   = sum_f sign(m1[p,f]-0.5)  (Sign w/ accum; nsr=(N+sgn)/2,
                                             0.5-ties fixed up on host)
        denA[p]  = sum md[p, 768:3328]      (Copy w/ accum, middle band)
  PE  : denP[s]  = sum md[p, outer bands]   (ones-column stationary matmuls
                                             accumulated into PSUM [4,512])
  DMA : m2 on the sync HWDGE queue, m1 on the gpsimd SWDGE queue (sample
        granularity, 8 KB lines); the queues round-robin at packet
        granularity for ~435 GB/s combined.

Host combine (exact, matches the reference's acc branch):
  den = denA + denP;  score = 2*(inter+1)/(den+1)
  corr_b = N - nSR - K + 2A with K (#elements == global max) and A (#those
  with m1 > 0.5) recovered by scanning only the partitions that attain the
  global max (O(2048) per sample, exact); score = 1 where corr == 1;
  loss = mean(1 - score)
"""

import os
import sys
import types

import numpy as np


def _ensure_concourse():
    try:
        import concourse.bass  # noqa: F401
    except ImportError:
        for p in ("/opt/trn_rl_repo", "/root/.axon_site/_ro/trn_rl_repo"):
            if os.path.isdir(p) and p not in sys.path:
                sys.path.insert(0, p)
        import concourse.bass  # noqa: F401


_ensure_concourse()

import concourse.bass as bass  # noqa: E402
import concourse.bacc as bacc  # noqa: E402
import concourse.tile as tile  # noqa: E402
from concourse import mybir  # noqa: E402
from concourse.bass_utils import run_bass_kernel_spmd  # noqa: E402
from concourse.vector_clock import ScopedClock  # noqa: E402

N_CORES = 8
B = 32                      # total batch
BPC = B // N_CORES          # samples per core
P = 128                     # partitions
F = 2048                    # free dim per partition (P*F = 512*512)

# den column split of the [m2|m1] combined tile (4096 cols):
# PE takes the outer bands [0, PE_LO) and [4096-PE_HI, 4096); ACT takes the
# middle band (contiguous for full-sample slots).
PE_LO = 896
PE_HI = 896
ACT_BAND = (PE_LO, 2 * F - PE_HI)

# sample chunking: samples 0-2 as full [128,2048] transfers, sample 3 split
# in two 1024-col chunks so the post-stream compute tail is shorter
SLOTS = [(0, 0, F), (1, 0, F), (2, 0, F), (3, 0, 1024), (3, 1024, 1024)]
NSL = len(SLOTS)


def _act_den_pieces(c0, w):
    """Contiguous md-column ranges the ACT den pass covers for slot (c0, w):
    ACT_BAND intersected with [c0, c0+w) u [F+c0, F+c0+w), adjacent ranges
    merged (full-sample slots collapse to one range across the m2|m1 seam)."""
    ranges = []
    for lo, hi in ((c0, c0 + w), (F + c0, F + c0 + w)):
        lo, hi = max(lo, ACT_BAND[0]), min(hi, ACT_BAND[1])
        if lo < hi:
            if ranges and ranges[-1][1] == lo:
                ranges[-1] = (ranges[-1][0], hi)
            else:
                ranges.append((lo, hi))
    return ranges


# PE matmul column ranges for slot (c0, w): outer-band pieces, <=512 wide
def _pe_mm_ranges(c0, w):
    out = []
    for lo, hi in ((c0, c0 + w), (F + c0, F + c0 + w)):
        lo2, hi2 = lo, min(hi, PE_LO) if lo < F else hi
        if lo < F:
            lo2, hi2 = lo, min(hi, PE_LO)
        else:
            lo2, hi2 = max(lo, 2 * F - PE_HI), hi
        q = lo2
        while q < hi2:
            qe = min(q + 512, hi2)
            out.append((q, qe))
            q = qe
    return out


def _slim_drain_and_barrier(self, tick_clock, wait_clock):
    # Same as TileContext._drain_and_barrier but without the second
    # all-engine barrier: NRT itself waits for every engine to halt before
    # the NEFF can be re-executed, so the sem clear does not need another
    # intra-NEFF barrier after it. (Bacc.compile legalizes multi-waits.)
    nc = self.nc
    drain_inst = nc.sync.drain()
    wait_clock.add_sem_waits(
        drain_inst.ins, ScopedClock({None: tick_clock.global_clock})
    )
    nc.all_engine_barrier()
    assert self.sems is not None
    popped = nc._tile_sem_poison_stack.pop()
    assert popped is self._sem_poison
    nc.clear_and_free_semaphores(list(self.sems.allocated().values()))


tile.TileContext._drain_and_barrier = _slim_drain_and_barrier


def _install_ntff_hook_module():
    """bass_utils imports antenv.axon_hooks when trace=True under axon; this
    container's antenv lacks that module. Recreate it from the boot helper."""
    if "antenv.axon_hooks" in sys.modules:
        return
    try:
        import trn_agent_boot.trn_boot as tb

        hook = tb._ntff_profile_via_ctypes("/opt/axon/libaxon_pjrt.so")
    except Exception:
        hook = None
    m = types.ModuleType("antenv.axon_hooks")
    m.get_axon_ntff_profile_hook = lambda: hook
    m.set_axon_ntff_profile_hook = lambda h: None
    sys.modules["antenv.axon_hooks"] = m


# stats tile layout (f32 columns)
COL_MAXP = 0
COL_INTER = NSL
COL_SGN = 2 * NSL
COL_DENA = 3 * NSL
# (slot_idx, md_lo, md_hi, stats_col) for every ACT den piece
DENA_PIECES = []
for _i, (_s, _c0, _w) in enumerate(SLOTS):
    for _lo, _hi in _act_den_pieces(_c0, _w):
        DENA_PIECES.append((_i, _lo, _hi, COL_DENA + len(DENA_PIECES)))
COL_DENP = COL_DENA + len(DENA_PIECES)  # rows 0..3: PE den partial per sample
N_COLS = COL_DENP + 1


def _build_nc():
    nc = bacc.Bacc("TRN2", debug=False)
    f32 = mybir.dt.float32
    probs = nc.dram_tensor("probs", [BPC, P, F], f32, kind="ExternalInput").ap()
    targets = nc.dram_tensor("targets", [BPC, P, F], f32, kind="ExternalInput").ap()
    stats_out = nc.dram_tensor("stats", [P, N_COLS], f32, kind="ExternalOutput").ap()

    A = mybir.AluOpType
    AF = mybir.ActivationFunctionType
    with tile.TileContext(nc) as tc:
        with (
            tc.tile_pool(name="inp", bufs=1) as inp_pool,
            tc.tile_pool(name="scr", bufs=1) as scr_pool,
            tc.tile_pool(name="stats", bufs=1) as stats_pool,
            tc.psum_pool(name="psum", bufs=1) as psum_pool,
        ):
            mds = [
                inp_pool.tile([P, 2 * F], f32, tag=f"md{s}", name=f"md{s}")
                for s in range(BPC)
            ]

            ones = scr_pool.tile([P, 4 * BPC], f32, tag="ones")
            neg_half = scr_pool.tile([P, 1], f32, tag="neg_half")
            dve_scr = scr_pool.tile([P, F], f32, tag="dve_scr")
            sgn_scr = scr_pool.tile([P, F], f32, tag="sgn_scr")
            cp_scr = scr_pool.tile([P, 2 * F - PE_LO - PE_HI], f32, tag="cp_scr")
            st = stats_pool.tile([P, N_COLS], f32, tag="st", name="st_all")
            psum_t = psum_pool.tile([BPC, 512], f32, tag="acc")

            # input DMAs, all on the one sync HWDGE queue (the sync engine is
            # otherwise idle, so the ~700ns dispatch cost is free); m2 into
            # the low half, m1 into the high half, interleaved per slot so
            # each sample's pair arrives together. 8KB per-partition lines.
            for s, c0, w in SLOTS:
                nc.sync.dma_start(
                    mds[s][:, c0 : c0 + w], targets[s][:, c0 : c0 + w]
                )
                nc.sync.dma_start(
                    mds[s][:, F + c0 : F + c0 + w], probs[s][:, c0 : c0 + w]
                )

            # stationary weights: cols 4s+s hold 1.0 so lhsT slice
            # [:, 4s:4s+4] routes sample s's column sums to PSUM row s
            nc.gpsimd.memset(ones[:], 0.0)
            for s in range(BPC):
                nc.gpsimd.memset(ones[:, 4 * s + s : 4 * s + s + 1], 1.0)
            nc.gpsimd.memset(neg_half[:], -0.5)

            all_mm = []
            for i, (s, c0, w) in enumerate(SLOTS):
                for q0, q1 in _pe_mm_ranges(c0, w):
                    all_mm.append((i, s, q0, q1))
            n_mm = 0
            pieces_by_slot = {}
            for i, lo, hi, col in DENA_PIECES:
                pieces_by_slot.setdefault(i, []).append((lo, hi, col))
            for i, (s, c0, w) in enumerate(SLOTS):
                md = mds[s]
                m2 = md[:, c0 : c0 + w]
                m1 = md[:, F + c0 : F + c0 + w]
                # DVE: per-partition max of targets
                nc.vector.tensor_reduce(
                    st[:, COL_MAXP + i : COL_MAXP + i + 1],
                    m2,
                    mybir.AxisListType.X,
                    A.max,
                )
                # ACT: count(m1 > 0.5) via sign(m1 - 0.5) accumulation
                nc.scalar.activation(
                    sgn_scr[:, 0:w], m1, AF.Sign, bias=neg_half[:],
                    accum_out=st[:, COL_SGN + i : COL_SGN + i + 1],
                )
                # DVE: intersection accumulate
                nc.vector.scalar_tensor_tensor(
                    out=dve_scr[:, 0:w],
                    in0=m1,
                    scalar=1.0,
                    in1=m2,
                    op0=A.mult,
                    op1=A.mult,
                    accum_out=st[:, COL_INTER + i : COL_INTER + i + 1],
                )
                # ACT: middle-band den pieces for this slot
                for lo, hi, col in pieces_by_slot.get(i, []):
                    nc.scalar.activation(
                        cp_scr[:, 0 : hi - lo],
                        md[:, lo:hi],
                        AF.Copy,
                        accum_out=st[:, col : col + 1],
                    )
                # PE: outer-band den matmuls for this slot
                for mi, ms, q0, q1 in all_mm:
                    if mi != i:
                        continue
                    nc.tensor.matmul(
                        psum_t[:, 0 : q1 - q0],
                        ones[:, 4 * ms : 4 * ms + 4],
                        md[:, q0:q1],
                        start=(n_mm == 0),
                        stop=(n_mm == len(all_mm) - 1),
                        skip_group_check=True,
                    )
                    n_mm += 1

            # DVE: fold PE partial sums -> per-sample den part in rows 0..3
            nc.vector.tensor_reduce(
                st[0:BPC, COL_DENP : COL_DENP + 1],
                psum_t[:, :],
                mybir.AxisListType.X,
                A.add,
            )

            nc.sync.dma_start(stats_out, st[:])

    nc.compile()
    return nc


def _shard_inputs(probs, targets):
    probs = np.ascontiguousarray(np.asarray(probs, dtype=np.float32)).reshape(B, P, F)
    targets = np.ascontiguousarray(np.asarray(targets, dtype=np.float32)).reshape(
        B, P, F
    )
    in_maps = []
    for i in range(N_CORES):
        sl = slice(i * BPC, (i + 1) * BPC)
        in_maps.append(
            {
                "probs": np.ascontiguousarray(probs[sl]),
                "targets": np.ascontiguousarray(targets[sl]),
            }
        )
    return in_maps


def _combine(results, probs, targets):
    """Exact host-side combine of per-partition stats -> scalar loss."""
    inter = np.empty(B)
    den = np.empty(B)
    corr = np.empty(B)
    N = float(P * F)
    for i in range(N_CORES):
        r = results[i]["stats"].astype(np.float64)
        for s in range(BPC):
            b = i * BPC + s
            idx = [j for j, (cs, _, _) in enumerate(SLOTS) if cs == s]
            inter[b] = r[:, [COL_INTER + j for j in idx]].sum()
            # den: ACT middle-band pieces of this sample's slots plus the
            # PE outer-band partial in row s
            dena_cols = [col for (j, _, _, col) in DENA_PIECES if j in idx]
            den[b] = r[:, dena_cols].sum() + r[s, COL_DENP]
            # nsr from sign sums; exact-0.5 ties make (w + S) odd -> rescan
            nsr = 0.0
            for j in idx:
                cs, c0, w = SLOTS[j]
                tot = w + r[:, COL_SGN + j]
                odd = np.nonzero(np.round(tot).astype(np.int64) & 1)[0]
                for p in odd:
                    tot[p] = 2 * np.count_nonzero(probs[b, p, c0 : c0 + w] > 0.5)
                nsr += tot.sum() / 2.0
            maxp = r[:, [COL_MAXP + j for j in idx]].max(axis=1)
            gmax = maxp.max()
            K = Acnt = 0
            for p in np.nonzero(maxp == gmax)[0]:
                hit = targets[b, p, :] == np.float32(gmax)
                K += int(hit.sum())
                Acnt += int((hit & (probs[b, p, :] > 0.5)).sum())
            corr[b] = N - nsr - K + 2 * Acnt
    score = 2.0 * (inter + 1.0) / (den + 1.0)
    score = np.where(corr == 1.0, 1.0, score)
    return np.array(np.mean(1.0 - score), dtype=np.float32)


def _run(probs, targets, trace=False, tmpdir=None):
    _install_ntff_hook_module()
    nc = _build_nc()
    in_maps = _shard_inputs(probs, targets)
    res = run_bass_kernel_spmd(
        nc, in_maps, list(range(N_CORES)), trace=trace, tmpdir=tmpdir
    )
    pr = np.asarray(probs, dtype=np.float32).reshape(B, P, F)
    tg = np.asarray(targets, dtype=np.float32).reshape(B, P, F)
    out = _combine(res.results, pr, tg)
    return out, res


def kernel(probs, targets):
    out, _ = _run(probs, targets)
    return out
